# revision 32
# baseline (speedup 1.0000x reference)
"""Query-chunk-parallel MultiHeadAttention kernel for 8 Trainium2 cores.

Problem: B=2, S=2048, D=512, H=8, per-head full-width projections.

Sharding: the B*S=4096 query rows split into 8 chunks of 512; chunk c
-> core c (b = c//4). Each core computes ALL 8 heads for its 512 query
rows and writes its own [512, D] slice of the final output: ZERO
collectives (the head-parallel variant spent ~70us in an exposed
ReduceScatter/AllGather tail plus a saturated cc stream).

Math restructuring (inherited from the verified head-parallel kernel):
  - softmax row-equivalences drop the K bias bk entirely; the V bias bv
    reduces to a constant row c = sum_h bv[h] @ Wo_h + bo added on the
    host at the end.
  - Host-fused weights (weight-weight products only):
      M_h = (Wq[h]/sqrt(D)) @ Wk[h]^T   so scores = q M_h k^T
      u_h = (bq[h]/sqrt(D)) @ Wk[h]^T   per-partition bias on QM^T
      W2_h = Wv[h] @ Wo_h               so out += (attn @ v) @ W2_h / den
    This removes the on-device K and V projections completely.
  - No softmax max-subtraction: score std ~0.33, |scores| < ~2.5.

Dataflow per head h (on this core's 512-row query chunk):
  QM^T[d2,m] = M_h^T q^T, +u, *64 -> fp8   (8 MM, fp8 DoubleRow)
  sT[k,m]    = k8 QM8                      (32 MM, fp8 DoubleRow)
               -> exp(ps/1024) on ACT -> PT (bf16)
  AT[d,m]    = v^T P (bf16)                (64 MM), den = DVE adds
  acc[m,e]  += (AT^T W2_h) / den_h         (16 MM + 2 DVE)
After head 7: acc (f32) -> bf16 -> DMA to the core's out slice.

Perf notes (from NTFF traces of the head-parallel ancestor):
  - Every 128-part x 512-free matmul costs ~263ns regardless of dtype
    (512 rows at the sustained-clock rate); fp8 DoubleRow wins by
    contracting 256 rows/MM, i.e. halved MM count, not faster MMs.
    960 MMs/core ~= 253us is the PE floor at these precisions.
  - Software pipelining: QM+scores of head h+1 interleave into the
    AV/out-projection of head h on the PE (ratio 2:1) so the exp (ACT)
    latency and QM->fp8 casts (DVE) never pace the tensor engine.
  - PE warm-up dummy matmuls keep the clock ramped through the initial
    DMA wait; loads are ordered first-use-first (wm_h0+q, k, v, ...).
"""
import os
import sys

sys.path.insert(0, "/opt/trn_rl_repo")
sys.path.insert(0, "/root/.axon_site")

import numpy as np

import concourse.bacc as bacc
import concourse.mybir as mybir
from concourse.tile import TileContext
from concourse import bass_utils

P = 128
B, S, D, H = 2, 2048, 512, 8
NCORES = 8
MC = 4               # query chunks per batch; B*MC == NCORES
CH = S // MC         # 512 query rows per core
DT = D // P          # 4 feature tiles
KT = S // P          # 16 k tiles (full batch seq per core)
F32 = mybir.dt.float32
BF16 = mybir.dt.bfloat16
FP8 = mybir.dt.float8e4

SK = 16.0            # host-side k scale into E4M3 range
SQ = 64.0            # device-side QM scale into E4M3 range
SQ2 = 16.0           # host-side q scale into E4M3 range

_NC_CACHE = {}

_SENT = object()


def _interleave(a_gen, b_gen, ratio_a=2):
    """Drain both generators; ratio_a steps of a per 1 of b while live."""
    a_live = b_live = True
    while a_live or b_live:
        if a_live:
            for _ in range(ratio_a):
                if next(a_gen, _SENT) is _SENT:
                    a_live = False
                    break
        if b_live and next(b_gen, _SENT) is _SENT:
            b_live = False


def _build_nc():
    nc = bacc.Bacc("TRN2", target_bir_lowering=False, debug=False,
                   num_devices=NCORES)

    # all inputs arrive in SBUF-image layout ([128, free...]) so DMA rows
    # are long and contiguous (2-32KB descriptors); small strided
    # descriptors (512B) measured ~4x below line rate
    qT8 = nc.dram_tensor("qT8", [P, DT, CH], FP8, kind="ExternalInput")
    kT8 = nc.dram_tensor("kT8", [P, DT, S], FP8, kind="ExternalInput")
    vn = nc.dram_tensor("vn", [P, KT, D], BF16, kind="ExternalInput")
    wm = nc.dram_tensor("wm", [P, H, DT, D], FP8, kind="ExternalInput")
    w2 = nc.dram_tensor("w2", [P, H, DT, D], BF16, kind="ExternalInput")
    uv = nc.dram_tensor("uv", [P, H, DT], F32, kind="ExternalInput")
    qmsc = nc.dram_tensor("qmsc", [P, H], F32, kind="ExternalInput")
    onesinv = nc.dram_tensor("onesinv", [P, 2], BF16, kind="ExternalInput")
    out = nc.dram_tensor("out", [CH, D], BF16, kind="ExternalOutput")

    with TileContext(nc) as tc:
        with (
            tc.tile_pool(name="consts", bufs=1) as consts,
            tc.tile_pool(name="qts", bufs=2) as qts,
            tc.tile_pool(name="pts", bufs=2) as pts,
            tc.tile_pool(name="small", bufs=3) as small,
            tc.tile_pool(name="accs", bufs=2) as accs,
            tc.tile_pool(name="ostage", bufs=4) as ostage,
            tc.tile_pool(name="rot", bufs=4, space="PSUM") as rot,
            tc.tile_pool(name="psout", bufs=1, space="PSUM") as psout,
        ):
            # ---- PE warm-up: tiny dummy matmuls keep the PE busy through
            # the initial DMA wait so the clock is ramped when the first
            # real matmul issues
            warm = consts.tile([P, 16], BF16, name="warm")
            wfull = consts.tile([P, 512], BF16, name="wfull")
            nc.vector.memset(warm[:], 1.0)
            nc.vector.memset(wfull[:], 1.0)
            wps = rot.tile([P, 512], F32, tag="ps")
            # tiny matmuls cover the DMA wait; the last few are full-width
            # so the PE power-state ramp (~3.5us of half-speed matmuls
            # otherwise) is absorbed before the first real matmul
            for _i in range(64):
                nc.tensor.matmul(wps[0:16, 0:16], lhsT=warm[:],
                                 rhs=warm[:, 0:16], start=True, stop=True)
            for _i in range(8):
                nc.tensor.matmul(wps[0:16, :], lhsT=warm[:],
                                 rhs=wfull[:], start=True, stop=True)

            # ---- SBUF-resident tensors, loaded in first-use order.
            wm_sb = consts.tile([P, H, DT, D], FP8, name="wm_sb")
            w2_sb = consts.tile([P, H, DT, D], BF16, name="w2_sb")
            q_sb = consts.tile([P, DT, CH], FP8, name="q_sb")
            k_sb = consts.tile([P, DT, S], FP8, name="k_sb")
            v_sb = consts.tile([P, KT, D], BF16, name="v_sb")
            u_sb = consts.tile([P, H, DT], F32, name="u_sb")
            qmsc_sb = consts.tile([P, H], F32, name="qmsc_sb")
            oinv_sb = consts.tile([P, 2], BF16, name="oinv_sb")
            acc = consts.tile([P, DT, CH], F32, name="acc")

            wm_ap = wm[:]
            w2_ap = w2[:]

            # three just-in-time DMA streams (sync / scalar / gpsimd),
            # issued in first-use order: QM ~7us needs q+wm0; scores
            # ~12us+ stream k tiles in order; AV ~24us+ stream v tiles;
            # QS(1) needs wm1, out-proj(0) needs w2_0.
            nc.sync.dma_start(wm_sb[:, 0], wm_ap[:, 0])
            nc.scalar.dma_start(q_sb[:], qT8[:])
            nc.gpsimd.dma_start(u_sb[:], uv[:])
            nc.gpsimd.dma_start(qmsc_sb[:], qmsc[:])
            nc.gpsimd.dma_start(oinv_sb[:], onesinv[:])
            # k: tiles 0-7 / 8-11 / 12-15 land in consumption order
            nc.sync.dma_start(k_sb[:, :, 0:1024], kT8[:, :, 0:1024])
            nc.scalar.dma_start(k_sb[:, :, 1024:1536], kT8[:, :, 1024:1536])
            nc.gpsimd.dma_start(k_sb[:, :, 1536:2048], kT8[:, :, 1536:2048])
            # v: tiles 0-5 / 6-10 / 11-15 in consumption order
            nc.sync.dma_start(v_sb[:, 0:6], vn[:, 0:6])
            nc.scalar.dma_start(v_sb[:, 6:11], vn[:, 6:11])
            nc.gpsimd.dma_start(v_sb[:, 11:16], vn[:, 11:16])
            nc.sync.dma_start(wm_sb[:, 1], wm_ap[:, 1])
            nc.scalar.dma_start(w2_sb[:, 0], w2_ap[:, 0])
            # wm[h>=2] / w2[h>=1] are deferred into the head loop below
            # (emitted from qs_gen(h>=1), which the pipeline only reaches
            # after real compute starts) so their descriptors don't
            # compete with q/k/v in the DMA rings

            state = {}

            def qs_gen(h):
                """QM projection + scores + exp for head h (yields per MM)."""
                # deferred weight streaming, ~1 phase ahead of use
                if h >= 1:
                    if h + 1 < H:
                        nc.gpsimd.dma_start(wm_sb[:, h + 1], wm_ap[:, h + 1])
                    nc.gpsimd.dma_start(w2_sb[:, h], w2_ap[:, h])
                QTc = qts.tile([P, DT, CH], FP8, tag="QT")
                PT = pts.tile([P, KT, CH], BF16, tag="PT")
                denA = small.tile([P, CH], F32, tag="denA")
                denBc = small.tile([P, CH], F32, tag="denBc")
                denB_sb = small.tile([P, CH], BF16, tag="denB_sb")
                state[h] = (QTc, PT, denB_sb)
                for et in range(DT):
                    ps = rot.tile([P, CH], F32, tag="ps")
                    for bq in range(2):
                        nc.tensor.matmul(
                            ps[:],
                            lhsT=wm_sb[:, h, 2 * bq:2 * bq + 2,
                                       et * P:(et + 1) * P],
                            rhs=q_sb[:, 2 * bq:2 * bq + 2, :],
                            start=(bq == 0), stop=(bq == 1),
                            perf_mode=mybir.MatmulPerfMode.DoubleRow,
                        )
                        yield
                    # QTc = (ps + u*sw*SQ2) * (SQ/(sw*SQ2)), per-head scale
                    nc.vector.tensor_scalar(
                        QTc[:, et, :], ps[:],
                        u_sb[:, h, et:et + 1], qmsc_sb[:, h:h + 1],
                        mybir.AluOpType.add, mybir.AluOpType.mult,
                    )
                for kt in range(KT):
                    ps = rot.tile([P, CH], F32, tag="ps")
                    for bk in range(2):
                        nc.tensor.matmul(
                            ps[:],
                            lhsT=k_sb[:, 2 * bk:2 * bk + 2,
                                      kt * P:(kt + 1) * P],
                            rhs=QTc[:, 2 * bk:2 * bk + 2, :],
                            start=(bk == 0), stop=(bk == 1),
                            perf_mode=mybir.MatmulPerfMode.DoubleRow,
                        )
                        yield
                    nc.scalar.activation(
                        PT[:, kt, :], ps[:],
                        mybir.ActivationFunctionType.Exp,
                        scale=1.0 / (SK * SQ),
                    )
                    # denominator chains chase the exps: even PT tiles on
                    # the otherwise-idle GpSimd, odd tiles on Vector, so
                    # den is ready when the out-projection needs recip and
                    # the av-phase Vector queue stays short
                    if kt == 2:
                        nc.gpsimd.tensor_add(denA[:], PT[:, 0, :],
                                             PT[:, 2, :])
                    elif kt == 3:
                        nc.vector.tensor_add(denBc[:], PT[:, 1, :],
                                             PT[:, 3, :])
                    elif kt >= 4:
                        if kt % 2 == 0:
                            nc.gpsimd.tensor_add(denA[:], denA[:],
                                                 PT[:, kt, :])
                        else:
                            nc.vector.tensor_add(denBc[:], denBc[:],
                                                 PT[:, kt, :])
                nc.vector.tensor_add(denB_sb[:], denA[:], denBc[:])

            def av_tail(h):
                """AV + out-projection + accumulate for head h."""
                QTc, PT, denB_sb = state.pop(h)

                outT_ps = psout.tile([P, DT, CH], F32, tag="outT")
                AT_sb = small.tile([P, DT, CH], BF16, tag="AT")
                recipT = small.tile([P, 8], F32, tag="recipT")
                for et in range(DT):
                    for kt in range(KT):
                        nc.tensor.matmul(
                            outT_ps[:, et, :],
                            lhsT=v_sb[:, kt, et * P:(et + 1) * P],
                            rhs=PT[:, kt, :],
                            start=(kt == 0), stop=(kt == KT - 1),
                        )
                        yield
                    nc.vector.tensor_copy(AT_sb[:, et, :], outT_ps[:, et, :])
                    if et == 2:
                        # den transpose + reciprocal after the 3rd AV
                        # tile: denB (chasing the exps) is long ready, no
                        # PE stall, and recip sits in the Vector queue
                        # ahead of the last AT cast so the
                        # out-projection's ACT scaling never waits
                        denT_ps = rot.tile([P, CH], F32, tag="ps")
                        for t in range(4):
                            nc.tensor.matmul(
                                denT_ps[:, 2 * t:2 * t + 2],
                                lhsT=denB_sb[:, t * P:(t + 1) * P],
                                rhs=oinv_sb[:],
                                start=True, stop=True,
                            )
                        yield
                        nc.vector.reciprocal(recipT[:], denT_ps[:, 0:8])
                # out-projection into the f32 accumulator (sum over heads)
                for t in range(4):
                    ps = rot.tile([P, CH], F32, tag="ps")
                    for et in range(DT):
                        nc.tensor.matmul(
                            ps[:],
                            lhsT=AT_sb[:, et, t * P:(t + 1) * P],
                            rhs=w2_sb[:, h, et, :],
                            start=(et == 0), stop=(et == DT - 1),
                        )
                        yield
                    # fused accumulate: acc = (ps * recip) + acc in ONE
                    # DVE op (scalar_tensor_tensor), minimizing the
                    # post-matmul chain at the kernel tail
                    rt = recipT[:, 2 * t:2 * t + 1]
                    if h == 0:
                        nc.vector.tensor_scalar_mul(acc[:, t, :], ps[:], rt)
                    elif h < H - 1:
                        nc.vector.scalar_tensor_tensor(
                            acc[:, t, :], ps[:], rt, acc[:, t, :],
                            mybir.AluOpType.mult, mybir.AluOpType.add,
                        )
                    else:
                        o_sb = ostage.tile([P, CH], BF16, tag="o")
                        nc.vector.scalar_tensor_tensor(
                            o_sb[:], ps[:], rt, acc[:, t, :],
                            mybir.AluOpType.mult, mybir.AluOpType.add,
                        )
                        eng = (nc.sync, nc.scalar, nc.sync, nc.scalar)[t]
                        eng.dma_start(out[t * P:(t + 1) * P, :], o_sb[:])

            # ---- software pipeline: QM+scores(h+1) hides inside AV(h)
            prev_tail = None
            for h in range(H):
                qs = qs_gen(h)
                if prev_tail is None:
                    for _ in qs:
                        pass
                else:
                    _interleave(prev_tail, qs, ratio_a=2)
                prev_tail = av_tail(h)
            for _ in prev_tail:
                pass

    nc.compile()
    return nc


def kernel(q, k, v, Wq, Wk, Wv, bq, bk, bv, Wo, bo):
    import ml_dtypes

    if "nc" not in _NC_CACHE:
        _NC_CACHE["nc"] = _build_nc()
    nc = _NC_CACHE["nc"]

    q = np.asarray(q, dtype=np.float32)
    k = np.asarray(k, dtype=np.float32)
    v = np.asarray(v, dtype=np.float32)
    Wq = np.asarray(Wq, dtype=np.float32)
    Wk = np.asarray(Wk, dtype=np.float32)
    Wv = np.asarray(Wv, dtype=np.float32)
    bq = np.asarray(bq, dtype=np.float32)
    bv = np.asarray(bv, dtype=np.float32)
    Wo = np.asarray(Wo, dtype=np.float32)
    bo = np.asarray(bo, dtype=np.float32)

    def cast16(x):
        return np.ascontiguousarray(
            np.asarray(x, dtype=np.float32).astype(ml_dtypes.bfloat16))

    def cast8(x, s):
        return np.ascontiguousarray(
            np.clip(np.asarray(x, np.float32) * s, -240.0, 240.0)
            .astype(ml_dtypes.float8_e4m3))

    scale = np.float32(1.0 / np.sqrt(D))

    # shared (replicated) weights, pre-shuffled into SBUF-image layout
    # ([P, ...free] with the partition index innermost-row) so device DMAs
    # move long contiguous rows
    wm_all = np.empty((H, D, D), dtype=ml_dtypes.float8_e4m3)
    w2_all = np.empty((H, D, D), dtype=ml_dtypes.bfloat16)
    uv_all = np.empty((H, D), dtype=np.float32)
    qmsc_all = np.empty((P, H), dtype=np.float32)
    for h in range(H):
        Wo_h = Wo[h * D:(h + 1) * D, :]
        wm_f = (Wq[h] * scale) @ Wk[h].T
        u_f = (bq[h] * scale) @ Wk[h].T
        # per-head power-of-2 weight scale into E4M3's normal range
        sw = float(2.0 ** np.floor(np.log2(
            128.0 / max(np.abs(wm_f).max(), 1e-30))))
        wm_all[h] = cast8(wm_f, sw)
        w2_all[h] = cast16(Wv[h] @ Wo_h)
        uv_all[h] = u_f * (sw * SQ2)
        qmsc_all[:, h] = SQ / (sw * SQ2)
    onesinv = cast16(np.ones((P, 2), dtype=np.float32))
    # [H, (DT P), e] -> [P, H, DT, e]
    wm_img = np.ascontiguousarray(
        wm_all.reshape(H, DT, P, D).transpose(2, 0, 1, 3))
    w2_img = np.ascontiguousarray(
        w2_all.reshape(H, DT, P, D).transpose(2, 0, 1, 3))
    uv_img = np.ascontiguousarray(
        uv_all.reshape(H, DT, P).transpose(2, 0, 1))

    def img3(x2d, inner):
        # [(n P), m] -> [P, n, m]
        n = x2d.shape[0] // P
        return np.ascontiguousarray(
            x2d.reshape(n, P, inner).transpose(1, 0, 2))

    in_maps = []
    for c in range(NCORES):
        b, qc = divmod(c, MC)
        in_maps.append({
            "qT8": img3(cast8(q[b].T[:, qc * CH:(qc + 1) * CH], SQ2), CH),
            "kT8": img3(cast8(k[b].T, SK), S),
            "vn": img3(cast16(v[b]), D),
            "wm": wm_img, "w2": w2_img, "uv": uv_img, "qmsc": qmsc_all,
            "onesinv": onesinv,
        })

    trace = bool(int(os.environ.get("KERNEL_TRACE", "0")))
    res = bass_utils.run_bass_kernel_spmd(
        nc, in_maps, core_ids=list(range(NCORES)), trace=trace
    )
    _NC_CACHE["last_result"] = res

    c_const = (sum(bv[h] @ Wo[h * D:(h + 1) * D, :] for h in range(H))
               + bo).astype(np.float32)
    out = np.empty((B, S, D), dtype=np.float32)
    for c in range(NCORES):
        b, qc = divmod(c, MC)
        out[b, qc * CH:(qc + 1) * CH, :] = (
            np.asarray(res.results[c]["out"], dtype=np.float32) + c_const)
    return out
